# revision 12
# baseline (speedup 1.0000x reference)
"""AttentionDecoderRNN forward step on 8 Trainium2 NeuronCores.

Strategy (zero cross-core communication):
- W_out (the 206MB output projection) is sharded over vocab across the 8
  cores; each core computes its logits slice with a multi-engine split:
  TensorE (fp32 matmuls over a host-transposed slice), VectorE/GpSimd
  (elementwise mult over the native-layout slice) and ScalarE (fused
  copy+accumulate acting as the row reduce).
- The attention + GRU chain is small and replicated on every core.
  Intermediate vectors live as partition-0 rows [1, n], column-chunk
  [128, k] tiles, or broadcast rows [128, n]; TensorE transposes and
  ones-broadcast matmuls convert between the forms.
- Each core also produces sum(exp(logits_slice)); the host combines the
  8 partial sums into logZ and finishes log_softmax while unsharding.
"""
import numpy as np

H = 1024
V = 50257
L = 512
N_CORES = 8
VS = 6400                 # per-core padded vocab shard (50 tiles of 128)
V_PAD = VS * N_CORES      # 51200
N_PE = 24                 # W_out tiles/core via TensorE (3072 rows)
N_NAT = 26                # W_out tiles/core via DVE/GP+ACT (3328 rows)
N_DVE6 = 16               # of N_NAT: DVE grp2 path (rest GpSimd+ACT)
COMB_PE = 512             # x rows via PE (rest native)
COMB_NAT = H - COMB_PE
GIPE = 512                # rows of the r-gate of W_ih on PE

_CACHE = {}


def _build():
    import concourse.bacc as bacc
    import concourse.mybir as mybir
    import concourse.tile as tile
    import concourse.bass_isa as bass_isa
    from concourse.masks import make_identity

    F32 = mybir.dt.float32
    AF = mybir.ActivationFunctionType
    OP = mybir.AluOpType

    nc = bacc.Bacc("TRN2", target_bir_lowering=False, debug=False,
                   num_devices=N_CORES)

    def inp(name, shape):
        return nc.dram_tensor(name, shape, F32, kind="ExternalInput").ap()

    def outp(name, shape):
        return nc.dram_tensor(name, shape, F32, kind="ExternalOutput").ap()

    # --- replicated chain inputs ---
    emb_row = inp("emb_row", [1, H])
    h0_cols = inp("h0_cols", [128, 8])
    h0_bc = inp("h0_bc", [128, H])
    cat1_cols = inp("cat1_cols", [128, 16])
    wt_attn = inp("wt_attn", [2 * H, L])          # W_attn.T
    b_attn_r = inp("b_attn_r", [1, L])
    e_nat = inp("e_nat", [L, H])                  # encoder_outputs (native)
    wt_comb = inp("wt_comb", [2 * H, COMB_PE])    # W_comb.T rows 0:COMB_PE
    wn_comb = inp("wn_comb", [COMB_NAT, 2 * H])   # native rows COMB_PE:
    b_comb_r = inp("b_comb_r", [1, COMB_PE])
    b_comb_c = inp("b_comb_c", [128, COMB_NAT // 128])
    wt_ih_pe = inp("wt_ih_pe", [H, 3 * GIPE])     # gate slices of W_ih.T
    wt_hh_pe = inp("wt_hh_pe", [H, 3 * GIPE])     # gate slices of W_hh.T
    w_ih_nat = inp("w_ih_nat", [3 * (H - GIPE), H])
    w_hh_nat = inp("w_hh_nat", [3 * (H - GIPE), H])
    b_ihhh_c = inp("b_ihhh_c", [128, 48])         # b_ih cols 24 | b_hh cols 24
    # --- sharded W_out ---
    wt_out = inp("wt_out", [H, N_PE * 128])
    wn_out = inp("wn_out", [N_NAT * 128, H])
    b_out_r = inp("b_out_r", [1, N_PE * 128])
    b_out_c = inp("b_out_c", [128, N_NAT])
    # --- outputs ---
    y_pe = outp("y_pe", [1, N_PE * 128])
    y_nat = outp("y_nat", [N_NAT, 128])
    y_s = outp("y_s", [1, 1])
    y_h = outp("y_h", [1, H])
    y_w = outp("y_w", [1, L])

    wn_out_t = wn_out.rearrange("(n p) h -> n p h", p=128)
    w_ih_nat_t = w_ih_nat.rearrange("(n p) h -> n p h", p=128)
    w_hh_nat_t = w_hh_nat.rearrange("(n p) h -> n p h", p=128)
    wn_comb_t = wn_comb.rearrange("(n p) h -> n p h", p=128)

    with tile.TileContext(nc) as tc:
        with (tc.tile_pool(name="wp", bufs=7) as wp,
              tc.tile_pool(name="dvp", bufs=4) as dvp,
              tc.tile_pool(name="gpp", bufs=4) as gpp,
              tc.tile_pool(name="hp", bufs=1) as hp,
              tc.tile_pool(name="sp", bufs=2) as sp,
              tc.tile_pool(name="ps", bufs=2, space="PSUM") as ps,
              tc.tile_pool(name="tpp", bufs=2, space="PSUM") as tpp,
              tc.tile_pool(name="psb", bufs=1, space="PSUM") as psb,
              tc.tile_pool(name="op", bufs=2) as op):

            # ---- constants / small loads ----
            r_emb = hp.tile([1, H], F32, tag="r_emb")
            nc.sync.dma_start(out=r_emb[:], in_=emb_row[:])
            c_h0 = hp.tile([128, 8], F32, tag="c_h0")
            nc.sync.dma_start(out=c_h0[:], in_=h0_cols[:])
            t_h0bc = hp.tile([128, H], F32, tag="t_h0bc")
            nc.sync.dma_start(out=t_h0bc[:], in_=h0_bc[:])
            c_cat1 = hp.tile([128, 16], F32, tag="c_cat1")
            nc.sync.dma_start(out=c_cat1[:], in_=cat1_cols[:])
            r_battn = hp.tile([1, L], F32, tag="r_battn")
            nc.sync.dma_start(out=r_battn[:], in_=b_attn_r[:])
            r_bcomb = hp.tile([1, COMB_PE], F32, tag="r_bcomb")
            nc.sync.dma_start(out=r_bcomb[:], in_=b_comb_r[:])
            c_bcomb = hp.tile([128, COMB_NAT // 128], F32, tag="c_bcomb")
            nc.sync.dma_start(out=c_bcomb[:], in_=b_comb_c[:])
            c_bg = hp.tile([128, 48], F32, tag="c_bg")
            nc.sync.dma_start(out=c_bg[:], in_=b_ihhh_c[:])
            r_bout = hp.tile([1, N_PE * 128], F32, tag="r_bout")
            nc.sync.dma_start(out=r_bout[:], in_=b_out_r[:])
            c_bout = hp.tile([128, N_NAT], F32, tag="c_bout")
            nc.sync.dma_start(out=c_bout[:], in_=b_out_c[:])
            ones_r = hp.tile([1, 128], F32, tag="ones_r")
            nc.vector.memset(ones_r[:], 1.0)
            ident = hp.tile([128, 128], F32, tag="ident")
            make_identity(nc, ident[:])

            # ---- helpers ----
            def dve_pair(w_aps, vec_bc, cols_dest2, ring, width):
                """Two native tiles fused: DVE mult + DVE reduce (grp=2)."""
                t = dvp.tile([128, 2, width], F32, tag="wd")
                ring.dma_start(out=t[:, 0, :], in_=w_aps[0])
                ring.dma_start(out=t[:, 1, :], in_=w_aps[1])
                prod = sp.tile([128, 2, width], F32, tag="pd")
                nc.vector.tensor_tensor(
                    out=prod[:], in0=t[:],
                    in1=vec_bc[:, None, :].broadcast_to([128, 2, width]),
                    op=OP.mult)
                nc.vector.tensor_reduce(out=cols_dest2, in_=prod[:],
                                        op=OP.add, axis=mybir.AxisListType.X)

            def dve_single(w_ap, vec_bc, col_dest, ring, width):
                """One native tile: DVE mult + DVE reduce (for wide tiles)."""
                t = dvp.tile([128, width], F32, tag="wd")
                ring.dma_start(out=t[:], in_=w_ap)
                prod = sp.tile([128, width], F32, tag="pd")
                nc.vector.tensor_tensor(out=prod[:], in0=t[:], in1=vec_bc[:],
                                        op=OP.mult)
                nc.vector.tensor_reduce(out=col_dest, in_=prod[:],
                                        op=OP.add, axis=mybir.AxisListType.X)

            def gp_tile(w_ap, vec_bc, col_dest, ring, width):
                """One native tile: GP mult + ACT accum-reduce (grp=1)."""
                t = gpp.tile([128, width], F32, tag="wg")
                ring.dma_start(out=t[:], in_=w_ap)
                prod = sp.tile([128, width], F32, tag="pg")
                nc.gpsimd.tensor_tensor(out=prod[:], in0=t[:], in1=vec_bc[:],
                                        op=OP.mult)
                nc.scalar.activation(prod[:], prod[:], AF.Identity,
                                     accum_out=col_dest)

            def nat_stage(w_view, n_tiles, width, vec_bc, cols_tile, c0,
                          n_dve, ring):
                # first n_dve tiles via DVE pairs, rest via GP singles
                g = 0
                while g < n_dve:
                    dve_pair([w_view[g], w_view[g + 1]], vec_bc,
                             cols_tile[:, c0 + g:c0 + g + 2], ring, width)
                    g += 2
                while g < n_tiles:
                    gp_tile(w_view[g], vec_bc,
                            cols_tile[:, c0 + g:c0 + g + 1], ring, width)
                    g += 1

            def row_to_cols(row_ap, cols_dest, k):
                for c in range(k):
                    tp = tpp.tile([128, 1], F32, tag="tp")
                    nc.tensor.transpose(tp[:], row_ap[:, 128 * c:128 * (c + 1)],
                                        ones_r[:, 0:1])
                    nc.vector.tensor_copy(cols_dest[:, c:c + 1], tp[:])

            def cols_to_row(cols_ap, row_dest, k, c0=0):
                for c in range(k):
                    tp = tpp.tile([1, 128], F32, tag="tpr")
                    nc.tensor.transpose(tp[:], cols_ap[:, c0 + c:c0 + c + 1],
                                        ident[:])
                    nc.scalar.copy(out=row_dest[:, 128 * c:128 * (c + 1)],
                                   in_=tp[:])

            def bcast_row_into(dest_bc, row_ap, n):
                for q in range(0, n, 512):
                    w = min(512, n - q)
                    bc_ps = psb.tile([128, 512], F32, tag="bc")
                    nc.tensor.matmul(bc_ps[:, 0:w], ones_r[:],
                                     row_ap[:, q:q + w],
                                     start=True, stop=True)
                    nc.vector.tensor_copy(dest_bc[:, q:q + w], bc_ps[:, 0:w])

            # =========================================================
            # S1: attn_logits = W_attn @ cat1 + b_attn   (PE only)
            # =========================================================
            aw_row = hp.tile([1, L], F32, tag="aw_row")
            acc1 = ps.tile([1, 512], F32, tag="acc")
            nc.tensor.matmul(acc1[:], ones_r[:, 0:1], r_battn[:],
                             start=True, stop=False)
            for c in range(16):
                t = wp.tile([128, L], F32, tag="w")
                ring = nc.sync if c % 2 == 0 else nc.scalar
                ring.dma_start(out=t[:],
                               in_=wt_attn[128 * c:128 * (c + 1), :])
                nc.tensor.matmul(acc1[:], c_cat1[:, c:c + 1], t[:],
                                 start=False, stop=(c == 15))
            # softmax over [1, 512]
            mx = hp.tile([1, 1], F32, tag="mx")
            nc.vector.tensor_reduce(out=mx[:], in_=acc1[:], op=OP.max,
                                    axis=mybir.AxisListType.X)
            nmx = hp.tile([1, 1], F32, tag="nmx")
            nc.vector.tensor_scalar_mul(nmx[:], mx[:], -1.0)
            nc.scalar.activation(aw_row[:], acc1[:], AF.Exp,
                                 bias=nmx[:], scale=1.0)
            sm = hp.tile([1, 1], F32, tag="sm")
            nc.vector.tensor_reduce(out=sm[:], in_=aw_row[:], op=OP.add,
                                    axis=mybir.AxisListType.X)
            rs = hp.tile([1, 1], F32, tag="rs")
            nc.vector.reciprocal(rs[:], sm[:])
            nc.vector.tensor_scalar_mul(aw_row[:], aw_row[:], rs[:])
            nc.gpsimd.dma_start(out=y_w[:], in_=aw_row[:])
            aw_cols = hp.tile([128, 4], F32, tag="aw_cols")
            row_to_cols(aw_row[:], aw_cols, 4)

            # =========================================================
            # S2: attn_applied = attn_w @ E  (PE, E native: contract L)
            # =========================================================
            aa_row = hp.tile([1, H], F32, tag="aa_row")
            for q in range(2):
                acc = ps.tile([1, 512], F32, tag="acc")
                for c in range(4):
                    t = wp.tile([128, 512], F32, tag="w")
                    ring = nc.sync if c % 2 == 0 else nc.scalar
                    ring.dma_start(
                        out=t[:], in_=e_nat[128 * c:128 * (c + 1),
                                            512 * q:512 * (q + 1)])
                    nc.tensor.matmul(acc[:], aw_cols[:, c:c + 1], t[:],
                                     start=(c == 0), stop=(c == 3))
                nc.scalar.copy(out=aa_row[:, 512 * q:512 * (q + 1)],
                               in_=acc[:])

            # cat2 row [1, 2048] and cols [128, 16]
            cat2_row = hp.tile([1, 2 * H], F32, tag="cat2_row")
            nc.scalar.copy(out=cat2_row[:, 0:H], in_=r_emb[:])
            nc.scalar.copy(out=cat2_row[:, H:2 * H], in_=aa_row[:])
            c_cat2 = hp.tile([128, 16], F32, tag="c_cat2")
            nc.vector.tensor_copy(c_cat2[:, 0:8], c_cat1[:, 0:8])
            row_to_cols(aa_row[:], c_cat2[:, 8:16], 8)
            cat2_bc = hp.tile([128, 2 * H], F32, tag="cat2_bc")
            bcast_row_into(cat2_bc, cat2_row[:], 2 * H)

            # =========================================================
            # S3: x = relu(W_comb @ cat2 + b_comb)
            # =========================================================
            acc2 = ps.tile([1, 512], F32, tag="acc")
            nc.tensor.matmul(acc2[:], ones_r[:, 0:1], r_bcomb[:],
                             start=True, stop=False)
            for c in range(16):
                t = wp.tile([128, COMB_PE], F32, tag="w")
                ring = nc.sync if c % 2 == 0 else nc.scalar
                ring.dma_start(out=t[:],
                               in_=wt_comb[128 * c:128 * (c + 1), :])
                nc.tensor.matmul(acc2[:], c_cat2[:, c:c + 1], t[:],
                                 start=False, stop=(c == 15))
            x_row = hp.tile([1, H], F32, tag="x_row")
            nc.scalar.activation(x_row[:, 0:COMB_PE], acc2[:], AF.Relu)
            xn_cols = hp.tile([128, COMB_NAT // 128], F32, tag="xn_cols")
            wn_comb_v = [wn_comb_t[g] for g in range(COMB_NAT // 128)]
            for g in range(COMB_NAT // 128):
                if g < 2:
                    dve_single(wn_comb_v[g], cat2_bc[:],
                               xn_cols[:, g:g + 1], nc.sync, 2 * H)
                else:
                    gp_tile(wn_comb_v[g], cat2_bc[:],
                            xn_cols[:, g:g + 1], nc.scalar, 2 * H)
            nc.vector.tensor_tensor(out=xn_cols[:], in0=xn_cols[:],
                                    in1=c_bcomb[:], op=OP.add)
            nc.scalar.activation(xn_cols[:], xn_cols[:], AF.Relu)
            cols_to_row(xn_cols[:], x_row[:, COMB_PE:], COMB_NAT // 128)
            x_bc = hp.tile([128, H], F32, tag="x_bc")
            bcast_row_into(x_bc, x_row[:], H)
            x_cols = hp.tile([128, 8], F32, tag="x_cols")
            row_to_cols(x_row[:], x_cols, 8)

            # =========================================================
            # S4: gi = W_ih @ x + b_ih ; gh = W_hh @ h0 + b_hh
            # gh native (sync ring); gi: PE part + native (scalar ring)
            # emitted interleaved so both rings stream in parallel.
            # =========================================================
            gi_cols = hp.tile([128, 24], F32, tag="gi_cols")
            gh_cols = hp.tile([128, 24], F32, tag="gh_cols")
            w_hh_v = [w_hh_nat_t[i] for i in range(12)]
            w_ih_v = [w_ih_nat_t[i] for i in range(12)]

            def nat_col(g):
                # native tile g (4 per gate) -> col index (second half of gate)
                return (g // 4) * 8 + 4 + (g % 4)

            def pe_gate_slice(wt_src, vec_cols, dest_cols, slice_i):
                acc = ps.tile([1, GIPE], F32, tag="acc")
                for c in range(8):
                    t = wp.tile([128, GIPE], F32, tag="w")
                    ring = nc.sync if c % 2 == 0 else nc.scalar
                    ring.dma_start(
                        out=t[:], in_=wt_src[128 * c:128 * (c + 1),
                                             GIPE * slice_i:
                                             GIPE * (slice_i + 1)])
                    nc.tensor.matmul(acc[:], vec_cols[:, c:c + 1], t[:],
                                     start=(c == 0), stop=(c == 7))
                grow = op.tile([1, GIPE], F32, tag="grow")
                nc.scalar.copy(out=grow[:], in_=acc[:])
                row_to_cols(grow[:], dest_cols[:, 8 * slice_i:
                                               8 * slice_i + 4], 4)

            # gh PE slices can run as soon as h0 is loaded
            for si in range(3):
                pe_gate_slice(wt_hh_pe, c_h0, gh_cols, si)
            for si in range(3):
                pe_gate_slice(wt_ih_pe, x_cols, gi_cols, si)
            # native halves: gh on sync ring, gi on scalar ring
            gh_d, gi_d = 0, 0
            while gh_d < 6 or gi_d < 6:
                if gh_d < 6:
                    dve_pair([w_hh_v[gh_d], w_hh_v[gh_d + 1]], t_h0bc[:],
                             gh_cols[:, nat_col(gh_d):nat_col(gh_d) + 2],
                             nc.sync, H)
                    gh_d += 2
                if gi_d < 6:
                    dve_pair([w_ih_v[gi_d], w_ih_v[gi_d + 1]], x_bc[:],
                             gi_cols[:, nat_col(gi_d):nat_col(gi_d) + 2],
                             nc.scalar, H)
                    gi_d += 2
            while gh_d < 12 or gi_d < 12:
                if gh_d < 12:
                    gp_tile(w_hh_v[gh_d], t_h0bc[:],
                            gh_cols[:, nat_col(gh_d):nat_col(gh_d) + 1],
                            nc.sync, H)
                    gh_d += 1
                if gi_d < 12:
                    gp_tile(w_ih_v[gi_d], x_bc[:],
                            gi_cols[:, nat_col(gi_d):nat_col(gi_d) + 1],
                            nc.scalar, H)
                    gi_d += 1

            nc.vector.tensor_tensor(out=gi_cols[:], in0=gi_cols[:],
                                    in1=c_bg[:, 0:24], op=OP.add)
            nc.vector.tensor_tensor(out=gh_cols[:], in0=gh_cols[:],
                                    in1=c_bg[:, 24:48], op=OP.add)

            # =========================================================
            # S5: GRU gates (col-chunk layout)
            # =========================================================
            rz = hp.tile([128, 16], F32, tag="rz")
            nc.vector.tensor_tensor(out=rz[:], in0=gi_cols[:, 0:16],
                                    in1=gh_cols[:, 0:16], op=OP.add)
            nc.scalar.activation(rz[:], rz[:], AF.Sigmoid)
            ng = hp.tile([128, 8], F32, tag="ng")
            nc.vector.tensor_tensor(out=ng[:], in0=rz[:, 0:8],
                                    in1=gh_cols[:, 16:24], op=OP.mult)
            nc.vector.tensor_tensor(out=ng[:], in0=ng[:],
                                    in1=gi_cols[:, 16:24], op=OP.add)
            nc.scalar.activation(ng[:], ng[:], AF.Tanh)
            hn_cols = hp.tile([128, 8], F32, tag="hn_cols")
            nc.vector.tensor_tensor(out=hn_cols[:], in0=c_h0[:],
                                    in1=ng[:], op=OP.subtract)
            nc.vector.tensor_tensor(out=hn_cols[:], in0=hn_cols[:],
                                    in1=rz[:, 8:16], op=OP.mult)
            nc.vector.tensor_tensor(out=hn_cols[:], in0=hn_cols[:],
                                    in1=ng[:], op=OP.add)
            hn_row = hp.tile([1, H], F32, tag="hn_row")
            cols_to_row(hn_cols[:], hn_row[:], 8)
            nc.gpsimd.dma_start(out=y_h[:], in_=hn_row[:])
            hn_bc = hp.tile([128, H], F32, tag="hn_bc")
            bcast_row_into(hn_bc, hn_row[:], H)

            # =========================================================
            # S6: logits = W_out_shard @ h_new + b_out; s = sum(exp(.))
            # =========================================================
            exp_parts = hp.tile([1, 8], F32, tag="exp_parts")
            nc.vector.memset(exp_parts[:], 0.0)
            cols = hp.tile([128, N_NAT], F32, tag="cols")
            npe_g = N_PE * 128 // 512

            def pe_group(g):
                acc = ps.tile([1, 512], F32, tag="acc")
                nc.tensor.matmul(acc[:], ones_r[:, 0:1],
                                 r_bout[:, 512 * g:512 * (g + 1)],
                                 start=True, stop=False)
                for c in range(8):
                    t = wp.tile([128, 512], F32, tag="w")
                    ring = nc.sync if c % 2 == 0 else nc.scalar
                    ring.dma_start(
                        out=t[:], in_=wt_out[128 * c:128 * (c + 1),
                                             512 * g:512 * (g + 1)])
                    nc.tensor.matmul(acc[:], hn_cols[:, c:c + 1], t[:],
                                     start=False, stop=(c == 7))
                row = op.tile([1, 512], F32, tag="row")
                nc.scalar.copy(out=row[:], in_=acc[:])
                nc.gpsimd.dma_start(out=y_pe[:, 512 * g:512 * (g + 1)],
                                    in_=row[:])
                erow = op.tile([1, 512], F32, tag="erow")
                nc.scalar.activation(erow[:], row[:], AF.Exp,
                                     accum_out=exp_parts[:, g:g + 1])

            wn_out_v = [wn_out_t[i] for i in range(N_NAT)]

            def nat_group6(g):
                if g < N_DVE6:
                    dve_pair([wn_out_v[g], wn_out_v[g + 1]], hn_bc[:],
                             cols[:, g:g + 2], nc.sync, H)
                    return 2
                gp_tile(wn_out_v[g], hn_bc[:], cols[:, g:g + 1],
                        nc.scalar, H)
                return 1

            pe_i, nat_i = 0, 0
            while pe_i < npe_g or nat_i < N_NAT:
                if pe_i < npe_g:
                    pe_group(pe_i)
                    pe_i += 1
                budget = 5
                while budget > 0 and nat_i < N_NAT:
                    nat_i += nat_group6(nat_i)
                    budget -= 1

            nc.vector.tensor_tensor(out=cols[:], in0=cols[:], in1=c_bout[:],
                                    op=OP.add)
            # exp-sum of native part
            ecols = sp.tile([128, N_NAT], F32, tag="pd")
            nc.scalar.activation(ecols[:], cols[:], AF.Exp)
            ecol1 = hp.tile([128, 1], F32, tag="ecol1")
            nc.vector.tensor_reduce(out=ecol1[:], in_=ecols[:], op=OP.add,
                                    axis=mybir.AxisListType.X)
            ecol_sum = hp.tile([128, 1], F32, tag="ecol_sum")
            nc.gpsimd.partition_all_reduce(ecol_sum[:], ecol1[:],
                                           channels=128,
                                           reduce_op=bass_isa.ReduceOp.add)
            s_all = hp.tile([1, 1], F32, tag="s_all")
            nc.vector.tensor_reduce(out=s_all[:], in_=exp_parts[:],
                                    op=OP.add, axis=mybir.AxisListType.X)
            nc.vector.tensor_tensor(out=s_all[:], in0=s_all[:],
                                    in1=ecol_sum[0:1, :], op=OP.add)
            nc.gpsimd.dma_start(out=y_s[:], in_=s_all[:])
            # native logits out: transpose cols -> rows
            colsT = ps.tile([N_NAT, 128], F32, tag="acc")
            nc.tensor.transpose(colsT[:], cols[:], ident[:])
            rowsn = op.tile([N_NAT, 128], F32, tag="rowsn")
            nc.scalar.copy(out=rowsn[:], in_=colsT[:])
            nc.gpsimd.dma_start(out=y_nat[:], in_=rowsn[:])
    nc.compile()
    return nc


def _cols8(v):
    return np.ascontiguousarray(np.asarray(v, np.float32).reshape(8, 128).T)


def _host_prep(input, hidden, encoder_outputs, emb, W_attn, b_attn,
               W_comb, b_comb, W_ih, W_hh, b_ih, b_hh, W_out, b_out):
    f32 = np.float32
    idx = int(np.asarray(input).ravel()[0])
    embedded = np.asarray(emb, f32)[idx]           # [H]
    h0 = np.asarray(hidden, f32).reshape(H)
    cat1 = np.concatenate([embedded, h0])

    W_attn = np.asarray(W_attn, f32)
    W_comb = np.asarray(W_comb, f32)
    W_ih = np.asarray(W_ih, f32)
    W_hh = np.asarray(W_hh, f32)
    W_out = np.asarray(W_out, f32)
    E = np.asarray(encoder_outputs, f32)

    # PE gate slices: first GIPE rows of each gate; rest native
    def gate_split(W):
        pe = np.concatenate([W[g * H:g * H + GIPE] for g in range(3)], axis=0)
        nat = np.concatenate([W[g * H + GIPE:(g + 1) * H]
                              for g in range(3)], axis=0)
        return pe, nat
    ih_pe, ih_nat = gate_split(W_ih)
    hh_pe, hh_nat = gate_split(W_hh)

    rep = {
        "emb_row": embedded.reshape(1, H).copy(),
        "h0_cols": _cols8(h0),
        "h0_bc": np.ascontiguousarray(np.broadcast_to(h0, (128, H))),
        "cat1_cols": np.ascontiguousarray(cat1.reshape(16, 128).T),
        "wt_attn": np.ascontiguousarray(W_attn.T),
        "b_attn_r": np.asarray(b_attn, f32).reshape(1, L),
        "e_nat": np.ascontiguousarray(E),
        "wt_comb": np.ascontiguousarray(W_comb[:COMB_PE].T),
        "wn_comb": np.ascontiguousarray(W_comb[COMB_PE:]),
        "b_comb_r": np.asarray(b_comb, f32)[:COMB_PE].reshape(1, -1),
        "b_comb_c": np.ascontiguousarray(
            np.asarray(b_comb, f32)[COMB_PE:].reshape(-1, 128).T),
        "wt_ih_pe": np.ascontiguousarray(ih_pe.T),
        "w_ih_nat": np.ascontiguousarray(ih_nat),
        "wt_hh_pe": np.ascontiguousarray(hh_pe.T),
        "w_hh_nat": np.ascontiguousarray(hh_nat),
        "b_ihhh_c": np.ascontiguousarray(np.concatenate(
            [np.asarray(b_ih, f32).reshape(24, 128).T,
             np.asarray(b_hh, f32).reshape(24, 128).T], axis=1)),
    }

    W_pad = np.zeros((V_PAD, H), f32)
    W_pad[:V] = W_out
    b_pad = np.full(V_PAD, -1e30, f32)
    b_pad[:V] = np.asarray(b_out, f32)

    in_maps = []
    for i in range(N_CORES):
        rows = W_pad[i * VS:(i + 1) * VS]
        brows = b_pad[i * VS:(i + 1) * VS]
        m = dict(rep)
        m["wt_out"] = np.ascontiguousarray(rows[:N_PE * 128].T)
        m["wn_out"] = np.ascontiguousarray(rows[N_PE * 128:])
        m["b_out_r"] = brows[:N_PE * 128].reshape(1, -1).copy()
        m["b_out_c"] = np.ascontiguousarray(
            brows[N_PE * 128:].reshape(N_NAT, 128).T)
        in_maps.append(m)
    return in_maps


def _unshard(results):
    f32 = np.float32
    parts = []
    s_tot = 0.0
    for r in results:
        parts.append(np.concatenate([r["y_pe"].reshape(-1),
                                     r["y_nat"].reshape(-1)]))
        s_tot += float(r["y_s"][0, 0])
    logits = np.concatenate(parts)[:V]
    logZ = np.log(s_tot)
    out = (logits - logZ).astype(f32)[None, :]
    h_new = results[0]["y_h"].reshape(1, 1, H).astype(f32)
    attn_w = results[0]["y_w"].reshape(1, L).astype(f32)
    return out, h_new, attn_w


def kernel(**inputs):
    from concourse.bass_utils import run_bass_kernel_spmd
    if "nc" not in _CACHE:
        _CACHE["nc"] = _build()
    nc = _CACHE["nc"]
    in_maps = _host_prep(**inputs)
    res = run_bass_kernel_spmd(nc, in_maps, list(range(N_CORES)))
    return _unshard(res.results)


# revision 13
# speedup vs baseline: 1.0526x; 1.0526x over previous
"""AttentionDecoderRNN forward step on 8 Trainium2 NeuronCores.

Strategy (zero cross-core communication):
- W_out (the 206MB output projection) is sharded over vocab across the 8
  cores; each core computes its logits slice with a multi-engine split:
  TensorE (fp32 matmuls over a host-transposed slice), VectorE/GpSimd
  (elementwise mult over the native-layout slice) and ScalarE (fused
  copy+accumulate acting as the row reduce).
- The attention + GRU chain is small and replicated on every core.
  Intermediate vectors live as partition-0 rows [1, n], column-chunk
  [128, k] tiles, or broadcast rows [128, n]; TensorE transposes and
  ones-broadcast matmuls convert between the forms.
- Each core also produces sum(exp(logits_slice)); the host combines the
  8 partial sums into logZ and finishes log_softmax while unsharding.
"""
import numpy as np

H = 1024
V = 50257
L = 512
N_CORES = 8
VS = 6400                 # per-core padded vocab shard (50 tiles of 128)
V_PAD = VS * N_CORES      # 51200
N_PE = 24                 # W_out tiles/core via TensorE (3072 rows)
N_NAT = 26                # W_out tiles/core via DVE/GP+ACT (3328 rows)
N_DVE6 = 18               # of N_NAT: DVE grp2 path (rest GpSimd+ACT)
COMB_PE = 512             # x rows via PE (rest native)
COMB_NAT = H - COMB_PE
GIPE = 512                # rows of the r-gate of W_ih on PE

_CACHE = {}


def _build():
    import concourse.bacc as bacc
    import concourse.mybir as mybir
    import concourse.tile as tile
    import concourse.bass_isa as bass_isa
    from concourse.masks import make_identity

    F32 = mybir.dt.float32
    AF = mybir.ActivationFunctionType
    OP = mybir.AluOpType

    nc = bacc.Bacc("TRN2", target_bir_lowering=False, debug=False,
                   num_devices=N_CORES)

    def inp(name, shape):
        return nc.dram_tensor(name, shape, F32, kind="ExternalInput").ap()

    def outp(name, shape):
        return nc.dram_tensor(name, shape, F32, kind="ExternalOutput").ap()

    # --- replicated chain inputs ---
    emb_row = inp("emb_row", [1, H])
    h0_cols = inp("h0_cols", [128, 8])
    h0_bc = inp("h0_bc", [128, H])
    cat1_cols = inp("cat1_cols", [128, 16])
    wt_attn = inp("wt_attn", [2 * H, L])          # W_attn.T
    b_attn_r = inp("b_attn_r", [1, L])
    e_nat = inp("e_nat", [L, H])                  # encoder_outputs (native)
    wt_comb = inp("wt_comb", [2 * H, COMB_PE])    # W_comb.T rows 0:COMB_PE
    wn_comb = inp("wn_comb", [COMB_NAT, 2 * H])   # native rows COMB_PE:
    b_comb_r = inp("b_comb_r", [1, COMB_PE])
    b_comb_c = inp("b_comb_c", [128, COMB_NAT // 128])
    wt_ih_pe = inp("wt_ih_pe", [H, 3 * GIPE])     # gate slices of W_ih.T
    wt_hh_pe = inp("wt_hh_pe", [H, 3 * GIPE])     # gate slices of W_hh.T
    w_ih_nat = inp("w_ih_nat", [3 * (H - GIPE), H])
    w_hh_nat = inp("w_hh_nat", [3 * (H - GIPE), H])
    b_ihhh_c = inp("b_ihhh_c", [128, 48])         # b_ih cols 24 | b_hh cols 24
    # --- sharded W_out ---
    wt_out = inp("wt_out", [H, N_PE * 128])
    wn_out = inp("wn_out", [N_NAT * 128, H])
    b_out_r = inp("b_out_r", [1, N_PE * 128])
    b_out_c = inp("b_out_c", [128, N_NAT])
    # --- outputs ---
    y_pe = outp("y_pe", [1, N_PE * 128])
    y_nat = outp("y_nat", [N_NAT, 128])
    y_s = outp("y_s", [1, 1])
    y_h = outp("y_h", [1, H])
    y_w = outp("y_w", [1, L])

    wn_out_t = wn_out.rearrange("(n p) h -> n p h", p=128)
    w_ih_nat_t = w_ih_nat.rearrange("(n p) h -> n p h", p=128)
    w_hh_nat_t = w_hh_nat.rearrange("(n p) h -> n p h", p=128)
    wn_comb_t = wn_comb.rearrange("(n p) h -> n p h", p=128)

    with tile.TileContext(nc) as tc:
        with (tc.tile_pool(name="wp", bufs=9) as wp,
              tc.tile_pool(name="dvp", bufs=5) as dvp,
              tc.tile_pool(name="gpp", bufs=4) as gpp,
              tc.tile_pool(name="hp", bufs=1) as hp,
              tc.tile_pool(name="sp", bufs=2) as sp,
              tc.tile_pool(name="ps", bufs=2, space="PSUM") as ps,
              tc.tile_pool(name="tpp", bufs=2, space="PSUM") as tpp,
              tc.tile_pool(name="psb", bufs=1, space="PSUM") as psb,
              tc.tile_pool(name="op", bufs=2) as op):

            # ---- constants / small loads ----
            r_emb = hp.tile([1, H], F32, tag="r_emb")
            nc.sync.dma_start(out=r_emb[:], in_=emb_row[:])
            c_h0 = hp.tile([128, 8], F32, tag="c_h0")
            nc.sync.dma_start(out=c_h0[:], in_=h0_cols[:])
            t_h0bc = hp.tile([128, H], F32, tag="bigbc1")
            nc.gpsimd.dma_start(out=t_h0bc[:], in_=h0_bc[:])
            c_cat1 = hp.tile([128, 16], F32, tag="c_cat1")
            nc.sync.dma_start(out=c_cat1[:], in_=cat1_cols[:])
            r_battn = hp.tile([1, L], F32, tag="r_battn")
            nc.sync.dma_start(out=r_battn[:], in_=b_attn_r[:])
            r_bcomb = hp.tile([1, COMB_PE], F32, tag="r_bcomb")
            nc.gpsimd.dma_start(out=r_bcomb[:], in_=b_comb_r[:])
            c_bcomb = hp.tile([128, COMB_NAT // 128], F32, tag="c_bcomb")
            nc.gpsimd.dma_start(out=c_bcomb[:], in_=b_comb_c[:])
            c_bg = hp.tile([128, 48], F32, tag="c_bg")
            nc.gpsimd.dma_start(out=c_bg[:], in_=b_ihhh_c[:])
            r_bout = hp.tile([1, N_PE * 128], F32, tag="r_bout")
            nc.gpsimd.dma_start(out=r_bout[:], in_=b_out_r[:])
            c_bout = hp.tile([128, N_NAT], F32, tag="c_bout")
            nc.gpsimd.dma_start(out=c_bout[:], in_=b_out_c[:])
            ones_r = hp.tile([1, 128], F32, tag="ones_r")
            nc.vector.memset(ones_r[:], 1.0)
            ident = hp.tile([128, 128], F32, tag="ident")
            make_identity(nc, ident[:])

            # ---- helpers ----
            def dve_pair(w_aps, vec_bc, cols_dest2, ring, width):
                """Two native tiles fused: DVE mult + DVE reduce (grp=2)."""
                t = dvp.tile([128, 2, width], F32, tag="wd")
                ring.dma_start(out=t[:, 0, :], in_=w_aps[0])
                ring.dma_start(out=t[:, 1, :], in_=w_aps[1])
                prod = sp.tile([128, 2, width], F32, tag="pd")
                nc.vector.tensor_tensor(
                    out=prod[:], in0=t[:],
                    in1=vec_bc[:, None, :].broadcast_to([128, 2, width]),
                    op=OP.mult)
                nc.vector.tensor_reduce(out=cols_dest2, in_=prod[:],
                                        op=OP.add, axis=mybir.AxisListType.X)

            def dve_single(w_ap, vec_bc, col_dest, ring, width):
                """One native tile: DVE mult + DVE reduce (for wide tiles)."""
                t = dvp.tile([128, width], F32, tag="wd")
                ring.dma_start(out=t[:], in_=w_ap)
                prod = sp.tile([128, width], F32, tag="pd")
                nc.vector.tensor_tensor(out=prod[:], in0=t[:], in1=vec_bc[:],
                                        op=OP.mult)
                nc.vector.tensor_reduce(out=col_dest, in_=prod[:],
                                        op=OP.add, axis=mybir.AxisListType.X)

            def gp_tile(w_ap, vec_bc, col_dest, ring, width):
                """One native tile: GP mult + ACT accum-reduce (grp=1)."""
                t = gpp.tile([128, width], F32, tag="wg")
                ring.dma_start(out=t[:], in_=w_ap)
                prod = sp.tile([128, width], F32, tag="pg")
                nc.gpsimd.tensor_tensor(out=prod[:], in0=t[:], in1=vec_bc[:],
                                        op=OP.mult)
                nc.scalar.activation(prod[:], prod[:], AF.Identity,
                                     accum_out=col_dest)

            def nat_stage(w_view, n_tiles, width, vec_bc, cols_tile, c0,
                          n_dve, ring):
                # first n_dve tiles via DVE pairs, rest via GP singles
                g = 0
                while g < n_dve:
                    dve_pair([w_view[g], w_view[g + 1]], vec_bc,
                             cols_tile[:, c0 + g:c0 + g + 2], ring, width)
                    g += 2
                while g < n_tiles:
                    gp_tile(w_view[g], vec_bc,
                            cols_tile[:, c0 + g:c0 + g + 1], ring, width)
                    g += 1

            def row_to_cols(row_ap, cols_dest, k):
                for c in range(k):
                    tp = tpp.tile([128, 1], F32, tag="tp")
                    nc.tensor.transpose(tp[:], row_ap[:, 128 * c:128 * (c + 1)],
                                        ones_r[:, 0:1])
                    nc.vector.tensor_copy(cols_dest[:, c:c + 1], tp[:])

            def cols_to_row(cols_ap, row_dest, k, c0=0):
                for c in range(k):
                    tp = tpp.tile([1, 128], F32, tag="tpr")
                    nc.tensor.transpose(tp[:], cols_ap[:, c0 + c:c0 + c + 1],
                                        ident[:])
                    nc.scalar.copy(out=row_dest[:, 128 * c:128 * (c + 1)],
                                   in_=tp[:])

            def bcast_row_into(dest_bc, row_ap, n):
                for q in range(0, n, 512):
                    w = min(512, n - q)
                    bc_ps = psb.tile([128, 512], F32, tag="bc")
                    nc.tensor.matmul(bc_ps[:, 0:w], ones_r[:],
                                     row_ap[:, q:q + w],
                                     start=True, stop=True)
                    nc.vector.tensor_copy(dest_bc[:, q:q + w], bc_ps[:, 0:w])

            # =========================================================
            # S1: attn_logits = W_attn @ cat1 + b_attn   (PE only)
            # =========================================================
            aw_row = hp.tile([1, L], F32, tag="aw_row")
            acc1 = ps.tile([1, 512], F32, tag="acc")
            nc.tensor.matmul(acc1[:], ones_r[:, 0:1], r_battn[:],
                             start=True, stop=False)
            for c in range(16):
                t = wp.tile([128, L], F32, tag="w")
                ring = nc.sync if c % 2 == 0 else nc.scalar
                ring.dma_start(out=t[:],
                               in_=wt_attn[128 * c:128 * (c + 1), :])
                nc.tensor.matmul(acc1[:], c_cat1[:, c:c + 1], t[:],
                                 start=False, stop=(c == 15))
            # softmax over [1, 512]
            mx = hp.tile([1, 1], F32, tag="mx")
            nc.vector.tensor_reduce(out=mx[:], in_=acc1[:], op=OP.max,
                                    axis=mybir.AxisListType.X)
            nmx = hp.tile([1, 1], F32, tag="nmx")
            nc.vector.tensor_scalar_mul(nmx[:], mx[:], -1.0)
            nc.scalar.activation(aw_row[:], acc1[:], AF.Exp,
                                 bias=nmx[:], scale=1.0)
            sm = hp.tile([1, 1], F32, tag="sm")
            nc.vector.tensor_reduce(out=sm[:], in_=aw_row[:], op=OP.add,
                                    axis=mybir.AxisListType.X)
            rs = hp.tile([1, 1], F32, tag="rs")
            nc.vector.reciprocal(rs[:], sm[:])
            nc.vector.tensor_scalar_mul(aw_row[:], aw_row[:], rs[:])
            nc.gpsimd.dma_start(out=y_w[:], in_=aw_row[:])
            aw_cols = hp.tile([128, 4], F32, tag="aw_cols")
            row_to_cols(aw_row[:], aw_cols, 4)

            # =========================================================
            # S2: attn_applied = attn_w @ E  (PE, E native: contract L)
            # =========================================================
            aa_row = hp.tile([1, H], F32, tag="aa_row")
            for q in range(2):
                acc = ps.tile([1, 512], F32, tag="acc")
                for c in range(4):
                    t = wp.tile([128, 512], F32, tag="w")
                    ring = nc.sync if c % 2 == 0 else nc.scalar
                    ring.dma_start(
                        out=t[:], in_=e_nat[128 * c:128 * (c + 1),
                                            512 * q:512 * (q + 1)])
                    nc.tensor.matmul(acc[:], aw_cols[:, c:c + 1], t[:],
                                     start=(c == 0), stop=(c == 3))
                nc.scalar.copy(out=aa_row[:, 512 * q:512 * (q + 1)],
                               in_=acc[:])

            # cat2 row [1, 2048] and cols [128, 16]
            cat2_row = hp.tile([1, 2 * H], F32, tag="cat2_row")
            nc.scalar.copy(out=cat2_row[:, 0:H], in_=r_emb[:])
            nc.scalar.copy(out=cat2_row[:, H:2 * H], in_=aa_row[:])
            c_cat2 = hp.tile([128, 16], F32, tag="c_cat2")
            nc.vector.tensor_copy(c_cat2[:, 0:8], c_cat1[:, 0:8])
            row_to_cols(aa_row[:], c_cat2[:, 8:16], 8)
            cat2_bc = hp.tile([128, 2 * H], F32, tag="cat2_bc")
            bcast_row_into(cat2_bc, cat2_row[:], 2 * H)

            # =========================================================
            # S3: x = relu(W_comb @ cat2 + b_comb)
            # =========================================================
            acc2 = ps.tile([1, 512], F32, tag="acc")
            nc.tensor.matmul(acc2[:], ones_r[:, 0:1], r_bcomb[:],
                             start=True, stop=False)
            for c in range(16):
                t = wp.tile([128, COMB_PE], F32, tag="w")
                ring = nc.sync if c % 2 == 0 else nc.scalar
                ring.dma_start(out=t[:],
                               in_=wt_comb[128 * c:128 * (c + 1), :])
                nc.tensor.matmul(acc2[:], c_cat2[:, c:c + 1], t[:],
                                 start=False, stop=(c == 15))
            x_row = hp.tile([1, H], F32, tag="x_row")
            nc.scalar.activation(x_row[:, 0:COMB_PE], acc2[:], AF.Relu)
            xn_cols = hp.tile([128, COMB_NAT // 128], F32, tag="xn_cols")
            wn_comb_v = [wn_comb_t[g] for g in range(COMB_NAT // 128)]
            for g in range(COMB_NAT // 128):
                if g < 2:
                    dve_single(wn_comb_v[g], cat2_bc[:],
                               xn_cols[:, g:g + 1], nc.sync, 2 * H)
                else:
                    gp_tile(wn_comb_v[g], cat2_bc[:],
                            xn_cols[:, g:g + 1], nc.scalar, 2 * H)
            nc.vector.tensor_tensor(out=xn_cols[:], in0=xn_cols[:],
                                    in1=c_bcomb[:], op=OP.add)
            nc.scalar.activation(xn_cols[:], xn_cols[:], AF.Relu)
            cols_to_row(xn_cols[:], x_row[:, COMB_PE:], COMB_NAT // 128)
            x_bc = hp.tile([128, H], F32, tag="x_bc")
            bcast_row_into(x_bc, x_row[:], H)
            x_cols = hp.tile([128, 8], F32, tag="x_cols")
            row_to_cols(x_row[:], x_cols, 8)

            # =========================================================
            # S4: gi = W_ih @ x + b_ih ; gh = W_hh @ h0 + b_hh
            # gh native (sync ring); gi: PE part + native (scalar ring)
            # emitted interleaved so both rings stream in parallel.
            # =========================================================
            gi_cols = hp.tile([128, 24], F32, tag="gi_cols")
            gh_cols = hp.tile([128, 24], F32, tag="gh_cols")
            w_hh_v = [w_hh_nat_t[i] for i in range(12)]
            w_ih_v = [w_ih_nat_t[i] for i in range(12)]

            def nat_col(g):
                # native tile g (4 per gate) -> col index (second half of gate)
                return (g // 4) * 8 + 4 + (g % 4)

            def pe_gate_slice(wt_src, vec_cols, dest_cols, slice_i):
                acc = ps.tile([1, GIPE], F32, tag="acc")
                for c in range(8):
                    t = wp.tile([128, GIPE], F32, tag="w")
                    ring = nc.sync if c % 2 == 0 else nc.scalar
                    ring.dma_start(
                        out=t[:], in_=wt_src[128 * c:128 * (c + 1),
                                             GIPE * slice_i:
                                             GIPE * (slice_i + 1)])
                    nc.tensor.matmul(acc[:], vec_cols[:, c:c + 1], t[:],
                                     start=(c == 0), stop=(c == 7))
                grow = op.tile([1, GIPE], F32, tag="grow")
                nc.scalar.copy(out=grow[:], in_=acc[:])
                row_to_cols(grow[:], dest_cols[:, 8 * slice_i:
                                               8 * slice_i + 4], 4)

            # gh PE slices can run as soon as h0 is loaded
            for si in range(3):
                pe_gate_slice(wt_hh_pe, c_h0, gh_cols, si)
            for si in range(3):
                pe_gate_slice(wt_ih_pe, x_cols, gi_cols, si)
            # native halves: gh on sync ring, gi on scalar ring
            gh_d, gi_d = 0, 0
            while gh_d < 6 or gi_d < 6:
                if gh_d < 6:
                    dve_pair([w_hh_v[gh_d], w_hh_v[gh_d + 1]], t_h0bc[:],
                             gh_cols[:, nat_col(gh_d):nat_col(gh_d) + 2],
                             nc.sync, H)
                    gh_d += 2
                if gi_d < 6:
                    dve_pair([w_ih_v[gi_d], w_ih_v[gi_d + 1]], x_bc[:],
                             gi_cols[:, nat_col(gi_d):nat_col(gi_d) + 2],
                             nc.scalar, H)
                    gi_d += 2
            while gh_d < 12 or gi_d < 12:
                if gh_d < 12:
                    gp_tile(w_hh_v[gh_d], t_h0bc[:],
                            gh_cols[:, nat_col(gh_d):nat_col(gh_d) + 1],
                            nc.sync, H)
                    gh_d += 1
                if gi_d < 12:
                    gp_tile(w_ih_v[gi_d], x_bc[:],
                            gi_cols[:, nat_col(gi_d):nat_col(gi_d) + 1],
                            nc.scalar, H)
                    gi_d += 1

            nc.vector.tensor_tensor(out=gi_cols[:], in0=gi_cols[:],
                                    in1=c_bg[:, 0:24], op=OP.add)
            nc.vector.tensor_tensor(out=gh_cols[:], in0=gh_cols[:],
                                    in1=c_bg[:, 24:48], op=OP.add)

            # =========================================================
            # S5: GRU gates (col-chunk layout)
            # =========================================================
            rz = hp.tile([128, 16], F32, tag="rz")
            nc.vector.tensor_tensor(out=rz[:], in0=gi_cols[:, 0:16],
                                    in1=gh_cols[:, 0:16], op=OP.add)
            nc.scalar.activation(rz[:], rz[:], AF.Sigmoid)
            ng = hp.tile([128, 8], F32, tag="ng")
            nc.vector.tensor_tensor(out=ng[:], in0=rz[:, 0:8],
                                    in1=gh_cols[:, 16:24], op=OP.mult)
            nc.vector.tensor_tensor(out=ng[:], in0=ng[:],
                                    in1=gi_cols[:, 16:24], op=OP.add)
            nc.scalar.activation(ng[:], ng[:], AF.Tanh)
            hn_cols = hp.tile([128, 8], F32, tag="hn_cols")
            nc.vector.tensor_tensor(out=hn_cols[:], in0=c_h0[:],
                                    in1=ng[:], op=OP.subtract)
            nc.vector.tensor_tensor(out=hn_cols[:], in0=hn_cols[:],
                                    in1=rz[:, 8:16], op=OP.mult)
            nc.vector.tensor_tensor(out=hn_cols[:], in0=hn_cols[:],
                                    in1=ng[:], op=OP.add)
            hn_row = hp.tile([1, H], F32, tag="hn_row")
            cols_to_row(hn_cols[:], hn_row[:], 8)
            nc.gpsimd.dma_start(out=y_h[:], in_=hn_row[:])
            hn_bc = hp.tile([128, H], F32, tag="bigbc1")
            bcast_row_into(hn_bc, hn_row[:], H)

            # =========================================================
            # S6: logits = W_out_shard @ h_new + b_out; s = sum(exp(.))
            # =========================================================
            exp_parts = hp.tile([1, 8], F32, tag="exp_parts")
            nc.vector.memset(exp_parts[:], 0.0)
            cols = hp.tile([128, N_NAT], F32, tag="cols")
            npe_g = N_PE * 128 // 512

            def pe_group(g):
                acc = ps.tile([1, 512], F32, tag="acc")
                nc.tensor.matmul(acc[:], ones_r[:, 0:1],
                                 r_bout[:, 512 * g:512 * (g + 1)],
                                 start=True, stop=False)
                for c in range(8):
                    t = wp.tile([128, 512], F32, tag="w")
                    ring = nc.sync if c % 2 == 0 else nc.scalar
                    ring.dma_start(
                        out=t[:], in_=wt_out[128 * c:128 * (c + 1),
                                             512 * g:512 * (g + 1)])
                    nc.tensor.matmul(acc[:], hn_cols[:, c:c + 1], t[:],
                                     start=False, stop=(c == 7))
                row = op.tile([1, 512], F32, tag="row")
                nc.scalar.copy(out=row[:], in_=acc[:])
                nc.gpsimd.dma_start(out=y_pe[:, 512 * g:512 * (g + 1)],
                                    in_=row[:])
                erow = op.tile([1, 512], F32, tag="erow")
                nc.scalar.activation(erow[:], row[:], AF.Exp,
                                     accum_out=exp_parts[:, g:g + 1])

            wn_out_v = [wn_out_t[i] for i in range(N_NAT)]

            def nat_group6(g):
                if g < N_DVE6:
                    dve_pair([wn_out_v[g], wn_out_v[g + 1]], hn_bc[:],
                             cols[:, g:g + 2], nc.sync, H)
                    return 2
                gp_tile(wn_out_v[g], hn_bc[:], cols[:, g:g + 1],
                        nc.scalar, H)
                return 1

            pe_i, nat_i = 0, 0
            while pe_i < npe_g or nat_i < N_NAT:
                if pe_i < npe_g:
                    pe_group(pe_i)
                    pe_i += 1
                budget = 5
                while budget > 0 and nat_i < N_NAT:
                    nat_i += nat_group6(nat_i)
                    budget -= 1

            nc.vector.tensor_tensor(out=cols[:], in0=cols[:], in1=c_bout[:],
                                    op=OP.add)
            # exp-sum of native part
            ecols = sp.tile([128, N_NAT], F32, tag="pd")
            nc.scalar.activation(ecols[:], cols[:], AF.Exp)
            ecol1 = hp.tile([128, 1], F32, tag="ecol1")
            nc.vector.tensor_reduce(out=ecol1[:], in_=ecols[:], op=OP.add,
                                    axis=mybir.AxisListType.X)
            ecol_sum = hp.tile([128, 1], F32, tag="ecol_sum")
            nc.gpsimd.partition_all_reduce(ecol_sum[:], ecol1[:],
                                           channels=128,
                                           reduce_op=bass_isa.ReduceOp.add)
            s_all = hp.tile([1, 1], F32, tag="s_all")
            nc.vector.tensor_reduce(out=s_all[:], in_=exp_parts[:],
                                    op=OP.add, axis=mybir.AxisListType.X)
            nc.vector.tensor_tensor(out=s_all[:], in0=s_all[:],
                                    in1=ecol_sum[0:1, :], op=OP.add)
            nc.gpsimd.dma_start(out=y_s[:], in_=s_all[:])
            # native logits out: transpose cols -> rows
            colsT = ps.tile([N_NAT, 128], F32, tag="acc")
            nc.tensor.transpose(colsT[:], cols[:], ident[:])
            rowsn = op.tile([N_NAT, 128], F32, tag="rowsn")
            nc.scalar.copy(out=rowsn[:], in_=colsT[:])
            nc.gpsimd.dma_start(out=y_nat[:], in_=rowsn[:])
    nc.compile()
    return nc


def _cols8(v):
    return np.ascontiguousarray(np.asarray(v, np.float32).reshape(8, 128).T)


def _host_prep(input, hidden, encoder_outputs, emb, W_attn, b_attn,
               W_comb, b_comb, W_ih, W_hh, b_ih, b_hh, W_out, b_out):
    f32 = np.float32
    idx = int(np.asarray(input).ravel()[0])
    embedded = np.asarray(emb, f32)[idx]           # [H]
    h0 = np.asarray(hidden, f32).reshape(H)
    cat1 = np.concatenate([embedded, h0])

    W_attn = np.asarray(W_attn, f32)
    W_comb = np.asarray(W_comb, f32)
    W_ih = np.asarray(W_ih, f32)
    W_hh = np.asarray(W_hh, f32)
    W_out = np.asarray(W_out, f32)
    E = np.asarray(encoder_outputs, f32)

    # PE gate slices: first GIPE rows of each gate; rest native
    def gate_split(W):
        pe = np.concatenate([W[g * H:g * H + GIPE] for g in range(3)], axis=0)
        nat = np.concatenate([W[g * H + GIPE:(g + 1) * H]
                              for g in range(3)], axis=0)
        return pe, nat
    ih_pe, ih_nat = gate_split(W_ih)
    hh_pe, hh_nat = gate_split(W_hh)

    rep = {
        "emb_row": embedded.reshape(1, H).copy(),
        "h0_cols": _cols8(h0),
        "h0_bc": np.ascontiguousarray(np.broadcast_to(h0, (128, H))),
        "cat1_cols": np.ascontiguousarray(cat1.reshape(16, 128).T),
        "wt_attn": np.ascontiguousarray(W_attn.T),
        "b_attn_r": np.asarray(b_attn, f32).reshape(1, L),
        "e_nat": np.ascontiguousarray(E),
        "wt_comb": np.ascontiguousarray(W_comb[:COMB_PE].T),
        "wn_comb": np.ascontiguousarray(W_comb[COMB_PE:]),
        "b_comb_r": np.asarray(b_comb, f32)[:COMB_PE].reshape(1, -1),
        "b_comb_c": np.ascontiguousarray(
            np.asarray(b_comb, f32)[COMB_PE:].reshape(-1, 128).T),
        "wt_ih_pe": np.ascontiguousarray(ih_pe.T),
        "w_ih_nat": np.ascontiguousarray(ih_nat),
        "wt_hh_pe": np.ascontiguousarray(hh_pe.T),
        "w_hh_nat": np.ascontiguousarray(hh_nat),
        "b_ihhh_c": np.ascontiguousarray(np.concatenate(
            [np.asarray(b_ih, f32).reshape(24, 128).T,
             np.asarray(b_hh, f32).reshape(24, 128).T], axis=1)),
    }

    W_pad = np.zeros((V_PAD, H), f32)
    W_pad[:V] = W_out
    b_pad = np.full(V_PAD, -1e30, f32)
    b_pad[:V] = np.asarray(b_out, f32)

    in_maps = []
    for i in range(N_CORES):
        rows = W_pad[i * VS:(i + 1) * VS]
        brows = b_pad[i * VS:(i + 1) * VS]
        m = dict(rep)
        m["wt_out"] = np.ascontiguousarray(rows[:N_PE * 128].T)
        m["wn_out"] = np.ascontiguousarray(rows[N_PE * 128:])
        m["b_out_r"] = brows[:N_PE * 128].reshape(1, -1).copy()
        m["b_out_c"] = np.ascontiguousarray(
            brows[N_PE * 128:].reshape(N_NAT, 128).T)
        in_maps.append(m)
    return in_maps


def _unshard(results):
    f32 = np.float32
    parts = []
    s_tot = 0.0
    for r in results:
        parts.append(np.concatenate([r["y_pe"].reshape(-1),
                                     r["y_nat"].reshape(-1)]))
        s_tot += float(r["y_s"][0, 0])
    logits = np.concatenate(parts)[:V]
    logZ = np.log(s_tot)
    out = (logits - logZ).astype(f32)[None, :]
    h_new = results[0]["y_h"].reshape(1, 1, H).astype(f32)
    attn_w = results[0]["y_w"].reshape(1, L).astype(f32)
    return out, h_new, attn_w


def kernel(**inputs):
    from concourse.bass_utils import run_bass_kernel_spmd
    if "nc" not in _CACHE:
        _CACHE["nc"] = _build()
    nc = _CACHE["nc"]
    in_maps = _host_prep(**inputs)
    res = run_bass_kernel_spmd(nc, in_maps, list(range(N_CORES)))
    return _unshard(res.results)


# revision 15
# speedup vs baseline: 1.1418x; 1.0847x over previous
"""AttentionDecoderRNN forward step on 8 Trainium2 NeuronCores.

Strategy (zero cross-core communication):
- W_out (the 206MB output projection) is sharded over vocab across the 8
  cores; each core computes its logits slice with a multi-engine split:
  TensorE (fp32 matmuls over a host-transposed slice), VectorE/GpSimd
  (elementwise mult over the native-layout slice) and ScalarE (fused
  copy+accumulate acting as the row reduce).
- The attention + GRU chain is small and replicated on every core.
  Intermediate vectors live as partition-0 rows [1, n], column-chunk
  [128, k] tiles, or broadcast rows [128, n]; TensorE transposes and
  ones-broadcast matmuls convert between the forms.
- Each core also produces sum(exp(logits_slice)); the host combines the
  8 partial sums into logZ and finishes log_softmax while unsharding.
"""
import numpy as np

H = 1024
V = 50257
L = 512
N_CORES = 8
VS = 6400                 # per-core padded vocab shard (50 tiles of 128)
V_PAD = VS * N_CORES      # 51200
N_PE = 24                 # W_out tiles/core via TensorE (3072 rows)
N_NAT = 26                # W_out tiles/core via DVE/GP+ACT (3328 rows)
N_DVE6 = 18               # of N_NAT: DVE grp2 path (rest GpSimd+ACT)
COMB_PE = 512             # x rows via PE (rest native)
COMB_NAT = H - COMB_PE
GIPE = 512                # rows of the r-gate of W_ih on PE

_CACHE = {}


def _build():
    import concourse.bacc as bacc
    import concourse.mybir as mybir
    import concourse.tile as tile
    import concourse.bass_isa as bass_isa
    from concourse.masks import make_identity

    F32 = mybir.dt.float32
    AF = mybir.ActivationFunctionType
    OP = mybir.AluOpType

    nc = bacc.Bacc("TRN2", target_bir_lowering=False, debug=False,
                   num_devices=N_CORES)

    def inp(name, shape):
        return nc.dram_tensor(name, shape, F32, kind="ExternalInput").ap()

    def outp(name, shape):
        return nc.dram_tensor(name, shape, F32, kind="ExternalOutput").ap()

    # --- replicated chain inputs ---
    emb_row = inp("emb_row", [1, H])
    h0_cols = inp("h0_cols", [128, 8])
    h0_bc = inp("h0_bc", [128, H])
    cat1_cols = inp("cat1_cols", [128, 16])
    wt_attn = inp("wt_attn", [2 * H, L])          # W_attn.T
    b_attn_r = inp("b_attn_r", [1, L])
    e_nat = inp("e_nat", [L, H])                  # encoder_outputs (native)
    wt_comb = inp("wt_comb", [2 * H, COMB_PE])    # W_comb.T rows 0:COMB_PE
    wn_comb = inp("wn_comb", [COMB_NAT, 2 * H])   # native rows COMB_PE:
    b_comb_r = inp("b_comb_r", [1, COMB_PE])
    b_comb_c = inp("b_comb_c", [128, COMB_NAT // 128])
    wt_ih_pe = inp("wt_ih_pe", [H, 3 * GIPE])     # gate slices of W_ih.T
    wt_hh_pe = inp("wt_hh_pe", [H, 3 * GIPE])     # gate slices of W_hh.T
    w_ih_nat = inp("w_ih_nat", [3 * (H - GIPE), H])
    w_hh_nat = inp("w_hh_nat", [3 * (H - GIPE), H])
    b_ihhh_c = inp("b_ihhh_c", [128, 48])         # b_ih cols 24 | b_hh cols 24
    # --- sharded W_out ---
    wt_out = inp("wt_out", [H, N_PE * 128])
    wn_out = inp("wn_out", [N_NAT * 128, H])
    b_out_r = inp("b_out_r", [1, N_PE * 128])
    b_out_c = inp("b_out_c", [128, N_NAT])
    # --- outputs ---
    y_pe = outp("y_pe", [1, N_PE * 128])
    y_nat = outp("y_nat", [N_NAT, 128])
    y_s = outp("y_s", [1, 1])
    y_h = outp("y_h", [1, H])
    y_w = outp("y_w", [1, L])

    wn_out_t = wn_out.rearrange("(n p) h -> n p h", p=128)
    w_ih_nat_t = w_ih_nat.rearrange("(n p) h -> n p h", p=128)
    w_hh_nat_t = w_hh_nat.rearrange("(n p) h -> n p h", p=128)
    wn_comb_t = wn_comb.rearrange("(n p) h -> n p h", p=128)

    with tile.TileContext(nc) as tc:
        with (tc.tile_pool(name="wp", bufs=9) as wp,
              tc.tile_pool(name="dvp", bufs=7) as dvp,
              tc.tile_pool(name="gpp", bufs=6) as gpp,
              tc.tile_pool(name="hp", bufs=1) as hp,
              tc.tile_pool(name="sp", bufs=2) as sp,
              tc.tile_pool(name="ps", bufs=2, space="PSUM") as ps,
              tc.tile_pool(name="tpp", bufs=2, space="PSUM") as tpp,
              tc.tile_pool(name="psb", bufs=1, space="PSUM") as psb,
              tc.tile_pool(name="op", bufs=2) as op):

            # ---- constants / small loads ----
            r_emb = hp.tile([1, H], F32, tag="r_emb")
            nc.sync.dma_start(out=r_emb[:], in_=emb_row[:])
            c_h0 = hp.tile([128, 8], F32, tag="c_h0")
            nc.sync.dma_start(out=c_h0[:], in_=h0_cols[:])
            t_h0bc = hp.tile([128, H], F32, tag="bigbc1")
            nc.gpsimd.dma_start(out=t_h0bc[:], in_=h0_bc[:])
            c_cat1 = hp.tile([128, 16], F32, tag="c_cat1")
            nc.sync.dma_start(out=c_cat1[:], in_=cat1_cols[:])
            r_battn = hp.tile([1, L], F32, tag="r_battn")
            nc.sync.dma_start(out=r_battn[:], in_=b_attn_r[:])
            r_bcomb = hp.tile([1, COMB_PE], F32, tag="r_bcomb")
            nc.gpsimd.dma_start(out=r_bcomb[:], in_=b_comb_r[:])
            c_bcomb = hp.tile([128, COMB_NAT // 128], F32, tag="c_bcomb")
            nc.gpsimd.dma_start(out=c_bcomb[:], in_=b_comb_c[:])
            c_bg = hp.tile([128, 48], F32, tag="c_bg")
            nc.gpsimd.dma_start(out=c_bg[:], in_=b_ihhh_c[:])
            r_bout = hp.tile([1, N_PE * 128], F32, tag="r_bout")
            nc.gpsimd.dma_start(out=r_bout[:], in_=b_out_r[:])
            c_bout = hp.tile([128, N_NAT], F32, tag="c_bout")
            nc.gpsimd.dma_start(out=c_bout[:], in_=b_out_c[:])
            ones_r = hp.tile([1, 128], F32, tag="ones_r")
            nc.vector.memset(ones_r[:], 1.0)
            ident = hp.tile([128, 128], F32, tag="ident")
            make_identity(nc, ident[:])
            warm = hp.tile([1, 4], F32, tag="warm")
            nc.scalar.activation(warm[:, 0:1], ones_r[:, 0:1], AF.Sigmoid)
            nc.scalar.activation(warm[:, 1:2], ones_r[:, 0:1], AF.Tanh)
            nc.scalar.activation(warm[:, 2:3], ones_r[:, 0:1], AF.Exp)

            # ---- helpers ----
            def dve_pair(w_aps, vec_bc, cols_dest2, ring, width):
                """Two native tiles fused: DVE mult + DVE reduce (grp=2)."""
                t = dvp.tile([128, 2, width], F32, tag="wd")
                ring.dma_start(out=t[:, 0, :], in_=w_aps[0])
                ring.dma_start(out=t[:, 1, :], in_=w_aps[1])
                prod = sp.tile([128, 2, width], F32, tag="pd")
                nc.vector.tensor_tensor(
                    out=prod[:], in0=t[:],
                    in1=vec_bc[:, None, :].broadcast_to([128, 2, width]),
                    op=OP.mult)
                nc.vector.tensor_reduce(out=cols_dest2, in_=prod[:],
                                        op=OP.add, axis=mybir.AxisListType.X)

            def dve_single(w_ap, vec_bc, col_dest, ring, width):
                """One native tile: DVE mult + DVE reduce (for wide tiles)."""
                t = dvp.tile([128, width], F32, tag="wd")
                ring.dma_start(out=t[:], in_=w_ap)
                prod = sp.tile([128, width], F32, tag="pd")
                nc.vector.tensor_tensor(out=prod[:], in0=t[:], in1=vec_bc[:],
                                        op=OP.mult)
                nc.vector.tensor_reduce(out=col_dest, in_=prod[:],
                                        op=OP.add, axis=mybir.AxisListType.X)

            def gp_tile(w_ap, vec_bc, col_dest, ring, width):
                """One native tile: GP mult + ACT accum-reduce (grp=1)."""
                t = gpp.tile([128, width], F32, tag="wg")
                ring.dma_start(out=t[:], in_=w_ap)
                prod = sp.tile([128, width], F32, tag="pg")
                nc.gpsimd.tensor_tensor(out=prod[:], in0=t[:], in1=vec_bc[:],
                                        op=OP.mult)
                nc.scalar.activation(prod[:], prod[:], AF.Identity,
                                     accum_out=col_dest)

            def nat_stage(w_view, n_tiles, width, vec_bc, cols_tile, c0,
                          n_dve, ring):
                # first n_dve tiles via DVE pairs, rest via GP singles
                g = 0
                while g < n_dve:
                    dve_pair([w_view[g], w_view[g + 1]], vec_bc,
                             cols_tile[:, c0 + g:c0 + g + 2], ring, width)
                    g += 2
                while g < n_tiles:
                    gp_tile(w_view[g], vec_bc,
                            cols_tile[:, c0 + g:c0 + g + 1], ring, width)
                    g += 1

            def row_to_cols(row_ap, cols_dest, k):
                for c in range(k):
                    tp = tpp.tile([128, 1], F32, tag="tp")
                    nc.tensor.transpose(tp[:], row_ap[:, 128 * c:128 * (c + 1)],
                                        ones_r[:, 0:1])
                    nc.vector.tensor_copy(cols_dest[:, c:c + 1], tp[:])

            def cols_to_row(cols_ap, row_dest, k, c0=0):
                for c in range(k):
                    tp = tpp.tile([1, 128], F32, tag="tpr")
                    nc.tensor.transpose(tp[:], cols_ap[:, c0 + c:c0 + c + 1],
                                        ident[:])
                    nc.scalar.copy(out=row_dest[:, 128 * c:128 * (c + 1)],
                                   in_=tp[:])

            def bcast_row_into(dest_bc, row_ap, n):
                for q in range(0, n, 512):
                    w = min(512, n - q)
                    bc_ps = psb.tile([128, 512], F32, tag="bc")
                    nc.tensor.matmul(bc_ps[:, 0:w], ones_r[:],
                                     row_ap[:, q:q + w],
                                     start=True, stop=True)
                    nc.vector.tensor_copy(dest_bc[:, q:q + w], bc_ps[:, 0:w])

            # =========================================================
            # S1: attn_logits = W_attn @ cat1 + b_attn   (PE only)
            # =========================================================
            aw_row = hp.tile([1, L], F32, tag="aw_row")
            acc1 = ps.tile([1, 512], F32, tag="acc")
            nc.tensor.matmul(acc1[:], ones_r[:, 0:1], r_battn[:],
                             start=True, stop=False)
            for c in range(16):
                t = wp.tile([128, L], F32, tag="w")
                ring = nc.sync if c % 2 == 0 else nc.scalar
                ring.dma_start(out=t[:],
                               in_=wt_attn[128 * c:128 * (c + 1), :])
                nc.tensor.matmul(acc1[:], c_cat1[:, c:c + 1], t[:],
                                 start=False, stop=(c == 15))
            # softmax over [1, 512]
            mx = hp.tile([1, 1], F32, tag="mx")
            nc.vector.tensor_reduce(out=mx[:], in_=acc1[:], op=OP.max,
                                    axis=mybir.AxisListType.X)
            nmx = hp.tile([1, 1], F32, tag="nmx")
            nc.vector.tensor_scalar_mul(nmx[:], mx[:], -1.0)
            nc.scalar.activation(aw_row[:], acc1[:], AF.Exp,
                                 bias=nmx[:], scale=1.0)
            sm = hp.tile([1, 1], F32, tag="sm")
            nc.vector.tensor_reduce(out=sm[:], in_=aw_row[:], op=OP.add,
                                    axis=mybir.AxisListType.X)
            rs = hp.tile([1, 1], F32, tag="rs")
            nc.vector.reciprocal(rs[:], sm[:])
            nc.vector.tensor_scalar_mul(aw_row[:], aw_row[:], rs[:])
            nc.gpsimd.dma_start(out=y_w[:], in_=aw_row[:])
            aw_cols = hp.tile([128, 4], F32, tag="aw_cols")
            row_to_cols(aw_row[:], aw_cols, 4)

            # =========================================================
            # S2: attn_applied = attn_w @ E  (PE, E native: contract L)
            # =========================================================
            aa_row = hp.tile([1, H], F32, tag="aa_row")
            for q in range(2):
                acc = ps.tile([1, 512], F32, tag="acc")
                for c in range(4):
                    t = wp.tile([128, 512], F32, tag="w")
                    ring = nc.sync if c % 2 == 0 else nc.scalar
                    ring.dma_start(
                        out=t[:], in_=e_nat[128 * c:128 * (c + 1),
                                            512 * q:512 * (q + 1)])
                    nc.tensor.matmul(acc[:], aw_cols[:, c:c + 1], t[:],
                                     start=(c == 0), stop=(c == 3))
                nc.scalar.copy(out=aa_row[:, 512 * q:512 * (q + 1)],
                               in_=acc[:])

            # cat2 row [1, 2048] and cols [128, 16]
            cat2_row = hp.tile([1, 2 * H], F32, tag="cat2_row")
            nc.scalar.copy(out=cat2_row[:, 0:H], in_=r_emb[:])
            nc.scalar.copy(out=cat2_row[:, H:2 * H], in_=aa_row[:])
            c_cat2 = hp.tile([128, 16], F32, tag="c_cat2")
            nc.vector.tensor_copy(c_cat2[:, 0:8], c_cat1[:, 0:8])
            row_to_cols(aa_row[:], c_cat2[:, 8:16], 8)
            cat2_bc = hp.tile([128, 2 * H], F32, tag="cat2_bc")
            bcast_row_into(cat2_bc, cat2_row[:], 2 * H)

            # =========================================================
            # S3: x = relu(W_comb @ cat2 + b_comb)
            # =========================================================
            acc2 = ps.tile([1, 512], F32, tag="acc")
            nc.tensor.matmul(acc2[:], ones_r[:, 0:1], r_bcomb[:],
                             start=True, stop=False)
            for c in range(16):
                t = wp.tile([128, COMB_PE], F32, tag="w")
                ring = nc.sync if c % 2 == 0 else nc.scalar
                ring.dma_start(out=t[:],
                               in_=wt_comb[128 * c:128 * (c + 1), :])
                nc.tensor.matmul(acc2[:], c_cat2[:, c:c + 1], t[:],
                                 start=False, stop=(c == 15))
            x_row = hp.tile([1, H], F32, tag="x_row")
            nc.scalar.activation(x_row[:, 0:COMB_PE], acc2[:], AF.Relu)
            xn_cols = hp.tile([128, COMB_NAT // 128], F32, tag="xn_cols")
            wn_comb_v = [wn_comb_t[g] for g in range(COMB_NAT // 128)]
            for g in range(COMB_NAT // 128):
                if g < 2:
                    dve_single(wn_comb_v[g], cat2_bc[:],
                               xn_cols[:, g:g + 1], nc.sync, 2 * H)
                else:
                    prod = sp.tile([128, 2, H], F32, tag="pd")
                    for half in range(2):
                        t = gpp.tile([128, H], F32, tag="wg")
                        nc.scalar.dma_start(
                            out=t[:], in_=wn_comb_v[g][:, H * half:
                                                       H * (half + 1)])
                        nc.gpsimd.tensor_tensor(
                            out=prod[:, half, :], in0=t[:],
                            in1=cat2_bc[:, H * half:H * (half + 1)],
                            op=OP.mult)
                    nc.scalar.activation(prod[:], prod[:], AF.Identity,
                                         accum_out=xn_cols[:, g:g + 1])
            nc.vector.tensor_tensor(out=xn_cols[:], in0=xn_cols[:],
                                    in1=c_bcomb[:], op=OP.add)
            nc.scalar.activation(xn_cols[:], xn_cols[:], AF.Relu)
            cols_to_row(xn_cols[:], x_row[:, COMB_PE:], COMB_NAT // 128)
            x_bc = hp.tile([128, H], F32, tag="x_bc")
            bcast_row_into(x_bc, x_row[:], H)
            x_cols = hp.tile([128, 8], F32, tag="x_cols")
            row_to_cols(x_row[:], x_cols, 8)

            # =========================================================
            # S4: gi = W_ih @ x + b_ih ; gh = W_hh @ h0 + b_hh
            # gh native (sync ring); gi: PE part + native (scalar ring)
            # emitted interleaved so both rings stream in parallel.
            # =========================================================
            hn_cols = hp.tile([128, 8], F32, tag="hn_cols")
            exp_parts = hp.tile([1, 8], F32, tag="exp_parts")
            nc.vector.memset(exp_parts[:], 0.0)
            cols = hp.tile([128, N_NAT], F32, tag="cols")
            npe_g = N_PE * 128 // 512

            def pe_group(g):
                acc = ps.tile([1, 512], F32, tag="acc")
                nc.tensor.matmul(acc[:], ones_r[:, 0:1],
                                 r_bout[:, 512 * g:512 * (g + 1)],
                                 start=True, stop=False)
                for c in range(8):
                    t = wp.tile([128, 512], F32, tag="w")
                    ring = nc.sync if c % 2 == 0 else nc.scalar
                    ring.dma_start(
                        out=t[:], in_=wt_out[128 * c:128 * (c + 1),
                                             512 * g:512 * (g + 1)])
                    nc.tensor.matmul(acc[:], hn_cols[:, c:c + 1], t[:],
                                     start=False, stop=(c == 7))
                row = op.tile([1, 512], F32, tag="row")
                nc.scalar.copy(out=row[:], in_=acc[:])
                nc.gpsimd.dma_start(out=y_pe[:, 512 * g:512 * (g + 1)],
                                    in_=row[:])
                erow = op.tile([1, 512], F32, tag="erow")
                nc.scalar.activation(erow[:], row[:], AF.Exp,
                                     accum_out=exp_parts[:, g:g + 1])
            gi_cols = hp.tile([128, 24], F32, tag="gi_cols")
            gh_cols = hp.tile([128, 24], F32, tag="gh_cols")
            w_hh_v = [w_hh_nat_t[i] for i in range(12)]
            w_ih_v = [w_ih_nat_t[i] for i in range(12)]

            def nat_col(g):
                # native tile g (4 per gate) -> col index (second half of gate)
                return (g // 4) * 8 + 4 + (g % 4)

            def pe_gate_slice(wt_src, vec_cols, dest_cols, slice_i):
                acc = ps.tile([1, GIPE], F32, tag="acc")
                for c in range(8):
                    t = wp.tile([128, GIPE], F32, tag="w")
                    ring = nc.sync if c % 2 == 0 else nc.scalar
                    ring.dma_start(
                        out=t[:], in_=wt_src[128 * c:128 * (c + 1),
                                             GIPE * slice_i:
                                             GIPE * (slice_i + 1)])
                    nc.tensor.matmul(acc[:], vec_cols[:, c:c + 1], t[:],
                                     start=(c == 0), stop=(c == 7))
                grow = op.tile([1, GIPE], F32, tag="grow")
                nc.scalar.copy(out=grow[:], in_=acc[:])
                row_to_cols(grow[:], dest_cols[:, 8 * slice_i:
                                               8 * slice_i + 4], 4)

            # gh PE slices can run as soon as h0 is loaded
            for si in range(3):
                pe_gate_slice(wt_hh_pe, c_h0, gh_cols, si)
            for si in range(3):
                pe_gate_slice(wt_ih_pe, x_cols, gi_cols, si)
            # native halves: gh on sync ring, gi on scalar ring
            gh_d, gi_d = 0, 0
            while gh_d < 6 or gi_d < 6:
                if gh_d < 6:
                    dve_pair([w_hh_v[gh_d], w_hh_v[gh_d + 1]], t_h0bc[:],
                             gh_cols[:, nat_col(gh_d):nat_col(gh_d) + 2],
                             nc.sync, H)
                    gh_d += 2
                if gi_d < 6:
                    dve_pair([w_ih_v[gi_d], w_ih_v[gi_d + 1]], x_bc[:],
                             gi_cols[:, nat_col(gi_d):nat_col(gi_d) + 2],
                             nc.scalar, H)
                    gi_d += 2
            while gh_d < 12 or gi_d < 12:
                if gh_d < 12:
                    gp_tile(w_hh_v[gh_d], t_h0bc[:],
                            gh_cols[:, nat_col(gh_d):nat_col(gh_d) + 1],
                            nc.sync, H)
                    gh_d += 1
                if gi_d < 12:
                    gp_tile(w_ih_v[gi_d], x_bc[:],
                            gi_cols[:, nat_col(gi_d):nat_col(gi_d) + 1],
                            nc.scalar, H)
                    gi_d += 1

            nc.vector.tensor_tensor(out=gi_cols[:], in0=gi_cols[:],
                                    in1=c_bg[:, 0:24], op=OP.add)
            nc.vector.tensor_tensor(out=gh_cols[:], in0=gh_cols[:],
                                    in1=c_bg[:, 24:48], op=OP.add)

            # =========================================================
            # S5: GRU gates (col-chunk layout)
            # =========================================================
            rz = hp.tile([128, 16], F32, tag="rz")
            nc.vector.tensor_tensor(out=rz[:], in0=gi_cols[:, 0:16],
                                    in1=gh_cols[:, 0:16], op=OP.add)
            nc.scalar.activation(rz[:], rz[:], AF.Sigmoid)
            ng = hp.tile([128, 8], F32, tag="ng")
            nc.vector.tensor_tensor(out=ng[:], in0=rz[:, 0:8],
                                    in1=gh_cols[:, 16:24], op=OP.mult)
            nc.vector.tensor_tensor(out=ng[:], in0=ng[:],
                                    in1=gi_cols[:, 16:24], op=OP.add)
            nc.scalar.activation(ng[:], ng[:], AF.Tanh)
            nc.vector.tensor_tensor(out=hn_cols[:], in0=c_h0[:],
                                    in1=ng[:], op=OP.subtract)
            nc.vector.tensor_tensor(out=hn_cols[:], in0=hn_cols[:],
                                    in1=rz[:, 8:16], op=OP.mult)
            nc.vector.tensor_tensor(out=hn_cols[:], in0=hn_cols[:],
                                    in1=ng[:], op=OP.add)
            hn_row = hp.tile([1, H], F32, tag="hn_row")
            cols_to_row(hn_cols[:], hn_row[:], 8)
            nc.gpsimd.dma_start(out=y_h[:], in_=hn_row[:])
            hn_bc = hp.tile([128, H], F32, tag="bigbc1")
            bcast_row_into(hn_bc, hn_row[:], H)

            # =========================================================
            # S6: logits = W_out_shard @ h_new + b_out; s = sum(exp(.))
            # =========================================================

            wn_out_v = [wn_out_t[i] for i in range(N_NAT)]

            def nat_group6(g):
                if g < N_DVE6:
                    dve_pair([wn_out_v[g], wn_out_v[g + 1]], hn_bc[:],
                             cols[:, g:g + 2], nc.sync, H)
                    return 2
                gp_tile(wn_out_v[g], hn_bc[:], cols[:, g:g + 1],
                        nc.scalar, H)
                return 1

            pe_i, nat_i = 0, 0
            while pe_i < npe_g or nat_i < N_NAT:
                if pe_i < npe_g:
                    pe_group(pe_i)
                    pe_i += 1
                budget = 5
                while budget > 0 and nat_i < N_NAT:
                    nat_i += nat_group6(nat_i)
                    budget -= 1

            nc.vector.tensor_tensor(out=cols[:], in0=cols[:], in1=c_bout[:],
                                    op=OP.add)
            # exp-sum of native part
            ecols = sp.tile([128, N_NAT], F32, tag="pd")
            nc.scalar.activation(ecols[:], cols[:], AF.Exp)
            ecol1 = hp.tile([128, 1], F32, tag="ecol1")
            nc.vector.tensor_reduce(out=ecol1[:], in_=ecols[:], op=OP.add,
                                    axis=mybir.AxisListType.X)
            ecol_sum = hp.tile([128, 1], F32, tag="ecol_sum")
            nc.gpsimd.partition_all_reduce(ecol_sum[:], ecol1[:],
                                           channels=128,
                                           reduce_op=bass_isa.ReduceOp.add)
            s_all = hp.tile([1, 1], F32, tag="s_all")
            nc.vector.tensor_reduce(out=s_all[:], in_=exp_parts[:],
                                    op=OP.add, axis=mybir.AxisListType.X)
            nc.vector.tensor_tensor(out=s_all[:], in0=s_all[:],
                                    in1=ecol_sum[0:1, :], op=OP.add)
            nc.gpsimd.dma_start(out=y_s[:], in_=s_all[:])
            # native logits out: transpose cols -> rows
            colsT = ps.tile([N_NAT, 128], F32, tag="acc")
            nc.tensor.transpose(colsT[:], cols[:], ident[:])
            rowsn = op.tile([N_NAT, 128], F32, tag="rowsn")
            nc.scalar.copy(out=rowsn[:], in_=colsT[:])
            nc.gpsimd.dma_start(out=y_nat[:], in_=rowsn[:])
    nc.compile()
    return nc


def _cols8(v):
    return np.ascontiguousarray(np.asarray(v, np.float32).reshape(8, 128).T)


def _host_prep(input, hidden, encoder_outputs, emb, W_attn, b_attn,
               W_comb, b_comb, W_ih, W_hh, b_ih, b_hh, W_out, b_out):
    f32 = np.float32
    idx = int(np.asarray(input).ravel()[0])
    embedded = np.asarray(emb, f32)[idx]           # [H]
    h0 = np.asarray(hidden, f32).reshape(H)
    cat1 = np.concatenate([embedded, h0])

    W_attn = np.asarray(W_attn, f32)
    W_comb = np.asarray(W_comb, f32)
    W_ih = np.asarray(W_ih, f32)
    W_hh = np.asarray(W_hh, f32)
    W_out = np.asarray(W_out, f32)
    E = np.asarray(encoder_outputs, f32)

    # PE gate slices: first GIPE rows of each gate; rest native
    def gate_split(W):
        pe = np.concatenate([W[g * H:g * H + GIPE] for g in range(3)], axis=0)
        nat = np.concatenate([W[g * H + GIPE:(g + 1) * H]
                              for g in range(3)], axis=0)
        return pe, nat
    ih_pe, ih_nat = gate_split(W_ih)
    hh_pe, hh_nat = gate_split(W_hh)

    rep = {
        "emb_row": embedded.reshape(1, H).copy(),
        "h0_cols": _cols8(h0),
        "h0_bc": np.ascontiguousarray(np.broadcast_to(h0, (128, H))),
        "cat1_cols": np.ascontiguousarray(cat1.reshape(16, 128).T),
        "wt_attn": np.ascontiguousarray(W_attn.T),
        "b_attn_r": np.asarray(b_attn, f32).reshape(1, L),
        "e_nat": np.ascontiguousarray(E),
        "wt_comb": np.ascontiguousarray(W_comb[:COMB_PE].T),
        "wn_comb": np.ascontiguousarray(W_comb[COMB_PE:]),
        "b_comb_r": np.asarray(b_comb, f32)[:COMB_PE].reshape(1, -1),
        "b_comb_c": np.ascontiguousarray(
            np.asarray(b_comb, f32)[COMB_PE:].reshape(-1, 128).T),
        "wt_ih_pe": np.ascontiguousarray(ih_pe.T),
        "w_ih_nat": np.ascontiguousarray(ih_nat),
        "wt_hh_pe": np.ascontiguousarray(hh_pe.T),
        "w_hh_nat": np.ascontiguousarray(hh_nat),
        "b_ihhh_c": np.ascontiguousarray(np.concatenate(
            [np.asarray(b_ih, f32).reshape(24, 128).T,
             np.asarray(b_hh, f32).reshape(24, 128).T], axis=1)),
    }

    W_pad = np.zeros((V_PAD, H), f32)
    W_pad[:V] = W_out
    b_pad = np.full(V_PAD, -1e30, f32)
    b_pad[:V] = np.asarray(b_out, f32)

    in_maps = []
    for i in range(N_CORES):
        rows = W_pad[i * VS:(i + 1) * VS]
        brows = b_pad[i * VS:(i + 1) * VS]
        m = dict(rep)
        m["wt_out"] = np.ascontiguousarray(rows[:N_PE * 128].T)
        m["wn_out"] = np.ascontiguousarray(rows[N_PE * 128:])
        m["b_out_r"] = brows[:N_PE * 128].reshape(1, -1).copy()
        m["b_out_c"] = np.ascontiguousarray(
            brows[N_PE * 128:].reshape(N_NAT, 128).T)
        in_maps.append(m)
    return in_maps


def _unshard(results):
    f32 = np.float32
    parts = []
    s_tot = 0.0
    for r in results:
        parts.append(np.concatenate([r["y_pe"].reshape(-1),
                                     r["y_nat"].reshape(-1)]))
        s_tot += float(r["y_s"][0, 0])
    logits = np.concatenate(parts)[:V]
    logZ = np.log(s_tot)
    out = (logits - logZ).astype(f32)[None, :]
    h_new = results[0]["y_h"].reshape(1, 1, H).astype(f32)
    attn_w = results[0]["y_w"].reshape(1, L).astype(f32)
    return out, h_new, attn_w


def kernel(**inputs):
    from concourse.bass_utils import run_bass_kernel_spmd
    if "nc" not in _CACHE:
        _CACHE["nc"] = _build()
    nc = _CACHE["nc"]
    in_maps = _host_prep(**inputs)
    res = run_bass_kernel_spmd(nc, in_maps, list(range(N_CORES)))
    return _unshard(res.results)
